# revision 19
# baseline (speedup 1.0000x reference)
"""DAGNN (gnn_message_passing) Trainium2 kernel — 8 NeuronCores.

Per core (dst-shard of N/8 nodes):
  * MLP (PE, bf16): x-shard -> h [16, SH] feature-major.
  * K propagation hops:
      - AllGather h shards (f16) -> gather table [128p = 8 src-eighths x 16
        feats, SH] (f32, cast during the table-load DMA)
      - GpSimd ap_gather per slot-segment (edges grouped by src-eighth, laid
        out in degree-sorted windows; width template shared across all
        (core, group) pairs so one tensor_reduce covers all 128 partitions)
      - DVE: multiply by uint8-quantized edge weights (resident in SBUF for
        all hops — no per-hop attr DMA) + windowed tensor_reduce -> L1
        partials (f32)
      - Realign+group-sum pipeline, chunked: GpSimd ap_gather (rank order ->
        dst order), Act scale-convert f32->f16 (dequant 1/(255*16)), PE
        matmul with a 0/1 selection matrix summing the 8 src-eighths,
        Act copy PSUM -> h' [16, SH] f16.
  * Combine: XBAR transpose-load of pps (f16), score = sigmoid(pps @ w + b),
    out = sum_k score_k * pps_k (f32).

h is rescaled by 1/16 per hop (folded into the dequant scale) to keep f16
pps in range; the combine phase multiplies 16^k back in f32.
"""

import math
import os
import sys

import numpy as np

try:
    import concourse  # noqa: F401
except ImportError:
    for _p in ("/opt/trn_rl_repo", "/root/.axon_site/_ro/trn_rl_repo"):
        if os.path.isdir(_p) and _p not in sys.path:
            sys.path.insert(0, _p)

import ml_dtypes

# problem sizes
N = 100000
E = 3200000
F_IN = 256
HID = 128
C = 16
K = 10
NCORES = 8
CHUNK_SLOTS = 3072          # max reduce segment size (slots)
GB = 4928                   # gather chunk scratch size (slots; 11 * RCHUNK)
RCHUNK = 448                # group-sum matmul chunk (<=512 for PSUM bank)
STG = 896                   # staging chunk (2 * RCHUNK)
AQ = 255.0                  # uint8 attr quantization scale

bf16 = ml_dtypes.bfloat16
f16 = np.float16


def _dims(n):
    sh = n // NCORES
    shp = ((sh + 127) // 128) * 128
    return sh, shp


# ----------------------------------------------------------------------------
# Host-side preprocessing
# ----------------------------------------------------------------------------

def _preprocess(edge_index, edge_attr):
    SH, SHP = _dims(N)
    src = np.asarray(edge_index[0], dtype=np.int64)
    dst = np.asarray(edge_index[1], dtype=np.int64)
    attr_q = np.clip(np.rint(np.asarray(edge_attr, np.float64) * AQ), 0,
                     255).astype(np.uint8)

    core_of = dst // SH
    grp_of = src // SH
    dloc = dst - core_of * SH
    sloc = src - grp_of * SH

    cg = (core_of * NCORES + grp_of).astype(np.int64)
    deg = np.zeros((NCORES * NCORES, SH), np.int64)
    np.add.at(deg, (cg, dloc), 1)
    deg = deg.reshape(NCORES, NCORES, SH)

    order = np.argsort(-deg, axis=2, kind="stable")          # [8,8,SH]
    rank = np.empty_like(order)
    ar = np.arange(SH)
    for c in range(NCORES):
        for g in range(NCORES):
            rank[c, g, order[c, g]] = ar
    sdeg = np.take_along_axis(deg, order, axis=2)
    W = np.maximum(sdeg.max(axis=(0, 1)), 1).astype(np.int64)  # non-increasing

    # segments: (w, n_windows, slot_off, l1_off); slot_off%16==0, n*w%16==0
    segs = []
    rank_l1 = np.zeros(SH, np.int64)
    slot_off = 0
    l1_off = 0
    bounds = [0] + list(np.flatnonzero(np.diff(W)) + 1) + [SH]
    for bi in range(len(bounds) - 1):
        i0, i1 = bounds[bi], bounds[bi + 1]
        w = int(W[i0])
        a = 32 // math.gcd(w, 32)
        per = max(a, (CHUNK_SLOTS // (a * w)) * a)
        assert per * w <= max(CHUNK_SLOTS, a * w)
        i = i0
        while i < i1:
            nn = min(per, i1 - i)
            n_pad = ((nn + a - 1) // a) * a
            rank_l1[i:i + nn] = l1_off + np.arange(nn)
            segs.append((w, n_pad, slot_off, l1_off))
            slot_off += n_pad * w
            l1_off += n_pad
            i += nn
    SLOTS = slot_off
    L1N = ((l1_off + 15) // 16) * 16
    assert SLOTS % 32 == 0 and L1N < 32768
    for (_w, _n, _so, _) in segs:
        assert _so % 32 == 0 and (_n * _w) % 32 == 0

    # greedy-pack gather chunks of <= GB slots, cutting inside segments at
    # window boundaries that land on 16-slot alignment
    cuts = [0]
    c0 = 0
    while c0 < SLOTS:
        lim = c0 + GB
        if lim >= SLOTS:
            cuts.append(SLOTS)
            break
        cut = None
        for (w, nn, so, lo) in segs:
            if so >= lim:
                break
            end = so + nn * w
            if end <= lim:
                cut = end
                continue
            # cut inside this segment (32-slot aligned: the gpsimd ucode
            # reads the wrapped idx columns as uint32 pairs)
            a32 = 32 // math.gcd(w, 32)
            m = ((lim - so) // w) // a32 * a32
            if m > 0:
                cut = so + m * w
            break
        assert cut is not None and cut > c0 and cut % 32 == 0
        cuts.append(cut)
        c0 = cut
    chunks = list(zip(cuts[:-1], cuts[1:]))

    seg_w = np.zeros(l1_off, np.int64)
    seg_so = np.zeros(l1_off, np.int64)
    seg_lo = np.zeros(l1_off, np.int64)
    for (w, nn, so, lo) in segs:
        seg_w[lo:lo + nn] = w
        seg_so[lo:lo + nn] = so
        seg_lo[lo:lo + nn] = lo
    slotstart = seg_so[rank_l1] + (rank_l1 - seg_lo[rank_l1]) * seg_w[rank_l1]

    gidx = np.zeros((NCORES, NCORES, SLOTS), np.int16)
    arep = np.zeros((NCORES, NCORES, SLOTS), np.uint8)
    e_sorted = np.argsort(cg, kind="stable")
    bnd = np.searchsorted(cg[e_sorted], np.arange(NCORES * NCORES + 1))
    for c in range(NCORES):
        for g in range(NCORES):
            kk = c * NCORES + g
            idx_e = e_sorted[bnd[kk]:bnd[kk + 1]]
            r = rank[c, g][dloc[idx_e]]
            o = np.argsort(r, kind="stable")
            idx_e, r = idx_e[o], r[o]
            startrun = np.flatnonzero(np.diff(r, prepend=-1))
            runlen = np.diff(np.append(startrun, len(r)))
            within = np.arange(len(r)) - np.repeat(startrun, runlen)
            pos = slotstart[r] + within
            gidx[c, g, pos] = sloc[idx_e].astype(np.int16)
            arep[c, g, pos] = attr_q[idx_e]

    ridx = np.zeros((NCORES, NCORES, SHP), np.int16)
    for c in range(NCORES):
        for g in range(NCORES):
            ridx[c, g, :SH] = rank_l1[rank[c, g]].astype(np.int16)

    def wrap(a):  # [8, L] -> [128, L//16] in ap_gather "(s p)" layout
        L = a.shape[-1]
        return np.ascontiguousarray(
            a.reshape(NCORES, L // 16, 16).transpose(0, 2, 1).reshape(128, L // 16))

    gidx_w = np.stack([wrap(gidx[c]) for c in range(NCORES)])
    ridx_w = np.stack([wrap(ridx[c]) for c in range(NCORES)])
    attr_r = np.repeat(arep[:, :, None, :], C, axis=2).reshape(
        NCORES, 128, SLOTS)

    return dict(segs=segs, chunks=chunks, SLOTS=SLOTS, L1N=L1N,
                gidx=gidx_w, ridx=ridx_w, attr=attr_r)


# ----------------------------------------------------------------------------
# Bass program
# ----------------------------------------------------------------------------

def _build_program(segs, chunks, SLOTS, L1N):
    import concourse.bass as bass
    import concourse.bacc as bacc
    import concourse.tile as tile
    from concourse import mybir

    SH, SHP = _dims(N)
    dt = mybir.dt
    AF = mybir.ActivationFunctionType
    ALU = mybir.AluOpType
    NSUBP = SHP // 128
    NRC = SHP // RCHUNK
    NSTG = SHP // STG
    assert NRC * RCHUNK == SHP and NSTG * STG == SHP
    assert GB % RCHUNK == 0 and SHP % RCHUNK == 0
    DEQ = 1.0 / (AQ * 16.0)

    nc = bacc.Bacc("TRN2", target_bir_lowering=False, debug=False,
                   num_devices=NCORES)

    xsh = nc.declare_dram_parameter("xsh", [SHP, F_IN], dt.float32, isOutput=False)
    w1t = nc.declare_dram_parameter("w1t", [F_IN, HID], dt.bfloat16, isOutput=False)
    w2t = nc.declare_dram_parameter("w2t", [HID, HID], dt.bfloat16, isOutput=False)
    w3t = nc.declare_dram_parameter("w3t", [HID, C], dt.bfloat16, isOutput=False)
    b1 = nc.declare_dram_parameter("b1", [HID, 1], dt.float32, isOutput=False)
    b2 = nc.declare_dram_parameter("b2", [HID, 1], dt.float32, isOutput=False)
    b3 = nc.declare_dram_parameter("b3", [C, 1], dt.float32, isOutput=False)
    ident = nc.declare_dram_parameter("ident", [128, 128], dt.bfloat16, isOutput=False)
    sel_p = nc.declare_dram_parameter("sel", [128, C], dt.float16, isOutput=False)
    gidx_d = nc.declare_dram_parameter("gidx", [128, SLOTS // 16], dt.int16, isOutput=False)
    ridx_d = nc.declare_dram_parameter("ridx", [128, SHP // 16], dt.int16, isOutput=False)
    attr_d = nc.declare_dram_parameter("attr", [128, SLOTS], dt.uint8, isOutput=False)
    wk_d = nc.declare_dram_parameter("wk", [128, K + 1, C], dt.float32, isOutput=False)
    sc_d = nc.declare_dram_parameter("sc", [128, K + 1], dt.float32, isOutput=False)
    pb_d = nc.declare_dram_parameter("pb", [128, 1], dt.float32, isOutput=False)
    out_d = nc.declare_dram_parameter("out", [SHP, C], dt.float32, isOutput=True)

    shard_d = nc.dram_tensor("shard_hbm", [C, SH], dt.float16)
    gath_d = nc.dram_tensor("gath_hbm", [NCORES, C, SH], dt.float16,
                            addr_space="Shared")
    pps_d = [nc.dram_tensor(f"pps{k}_hbm", [C, SHP], dt.float16)
             for k in range(K + 1)]

    groups = [list(range(NCORES))]

    with tile.TileContext(nc) as tc:
        with tc.tile_pool(name="const", bufs=1) as constp:
            gidx_sb = constp.tile([128, SLOTS // 16], dt.int16)
            nc.sync.dma_start(out=gidx_sb[:], in_=gidx_d[:])
            ridx_sb = constp.tile([128, SHP // 16], dt.int16)
            nc.sync.dma_start(out=ridx_sb[:], in_=ridx_d[:])
            attr_sb = constp.tile([128, SLOTS], dt.uint8)
            nc.scalar.dma_start(out=attr_sb[:], in_=attr_d[:])
            sel_sb = constp.tile([128, C], dt.float16)
            nc.sync.dma_start(out=sel_sb[:], in_=sel_p[:])

            # ---------- MLP ----------
            with (
                tc.tile_pool(name="mlp", bufs=2) as mlpp,
                tc.tile_pool(name="mlpc", bufs=1) as mlpc,
                tc.tile_pool(name="mpsum", bufs=2, space="PSUM") as mpsum,
            ):
                ident_sb = mlpc.tile([128, 128], dt.bfloat16)
                nc.sync.dma_start(out=ident_sb[:], in_=ident[:])
                w1_sb = mlpc.tile([128, 2, HID], dt.bfloat16)
                nc.sync.dma_start(out=w1_sb[:],
                                  in_=w1t.rearrange("(a p) m -> p a m", p=128))
                w2_sb = mlpc.tile([HID, HID], dt.bfloat16)
                nc.sync.dma_start(out=w2_sb[:], in_=w2t[:])
                w3_sb = mlpc.tile([HID, C], dt.bfloat16)
                nc.sync.dma_start(out=w3_sb[:], in_=w3t[:])
                b1_sb = mlpc.tile([HID, 1], dt.float32)
                nc.sync.dma_start(out=b1_sb[:], in_=b1[:])
                b2_sb = mlpc.tile([HID, 1], dt.float32)
                nc.sync.dma_start(out=b2_sb[:], in_=b2[:])
                b3_sb = mlpc.tile([C, 1], dt.float32)
                nc.sync.dma_start(out=b3_sb[:], in_=b3[:])
                hsb = mlpc.tile([C, SHP], dt.float16)

                coff = 0
                ci = 0
                while coff < SHP:
                    cn = min(512, SHP - coff)
                    nsub = cn // 128
                    xin = mlpp.tile([128, nsub, F_IN], dt.float32, tag="xin")
                    eng = nc.sync if ci % 2 == 0 else nc.scalar
                    eng.dma_start(
                        out=xin[:],
                        in_=xsh.rearrange("(a p) f -> p a f", p=128)[
                            :, coff // 128:coff // 128 + nsub, :])
                    xbf = mlpp.tile([128, nsub, F_IN], dt.bfloat16, tag="xbf")
                    nc.vector.tensor_copy(xbf[:], xin[:])
                    xT = mlpp.tile([128, 2, cn], dt.bfloat16, tag="xT")
                    for s in range(nsub):
                        for hlf in range(2):
                            tp = mpsum.tile([128, 128], dt.bfloat16, tag="tp")
                            nc.tensor.transpose(
                                tp[:], xbf[:, s, hlf * 128:(hlf + 1) * 128],
                                ident_sb[:])
                            nc.scalar.activation(
                                xT[:, hlf, s * 128:(s + 1) * 128], tp[:], AF.Copy)
                    h1p = mpsum.tile([HID, cn], dt.float32, tag="h1p")
                    nc.tensor.matmul(h1p[:], w1_sb[:, 0, :], xT[:, 0, :],
                                     start=True, stop=False)
                    nc.tensor.matmul(h1p[:], w1_sb[:, 1, :], xT[:, 1, :],
                                     start=False, stop=True)
                    x2 = mlpp.tile([HID, cn], dt.bfloat16, tag="x2")
                    nc.scalar.activation(x2[:], h1p[:], AF.Relu, bias=b1_sb[:])
                    h2p = mpsum.tile([HID, cn], dt.float32, tag="h1p")
                    nc.tensor.matmul(h2p[:], w2_sb[:], x2[:], start=True, stop=True)
                    x3 = mlpp.tile([HID, cn], dt.bfloat16, tag="x2")
                    nc.scalar.activation(x3[:], h2p[:], AF.Relu, bias=b2_sb[:])
                    h3p = mpsum.tile([C, cn], dt.float32, tag="h3p")
                    nc.tensor.matmul(h3p[:], w3_sb[:], x3[:], start=True, stop=True)
                    nc.scalar.activation(hsb[:, coff:coff + cn], h3p[:], AF.Relu,
                                         bias=b3_sb[:])
                    coff += cn
                    ci += 1
                nc.sync.dma_start(out=pps_d[0][:], in_=hsb[:])
                nc.sync.dma_start(out=shard_d[:], in_=hsb[:, 0:SH])

            # ---------- propagation hops ----------
            with (
                tc.tile_pool(name="pers", bufs=1) as pers,
                tc.tile_pool(name="hopp", bufs=2) as hopp,
                tc.tile_pool(name="alp", bufs=2) as alp,
                tc.tile_pool(name="hpsum", bufs=4, space="PSUM") as hpsum,
            ):
                TBP = ((SH + 3) // 4) * 4 + 4
                table = pers.tile([128, TBP, 1], dt.float32)
                l1out = pers.tile([128, L1N, 1], dt.float32)
                # f16 staging aliased onto the upper half of the table buffer;
                # the in-place expanding cast is safe because the f32 write
                # front always trails the f16 read position
                tabh = table[:, TBP // 2:TBP, 0].bitcast(dt.float16)

                gath_r = gath_d.rearrange("a b c -> (a b) c")
                for k in range(1, K + 1):
                    nc.gpsimd.collective_compute(
                        "AllGather", ALU.bypass, replica_groups=groups,
                        ins=[shard_d.ap()], outs=[gath_d.ap()])
                    # raw f16 table load on the HWDGE queues (a casting DMA
                    # would run the f16->f32 convert as a ~130us Q7 software
                    # loop, stalling the gathers); cast on Act instead
                    nc.sync.dma_start(out=tabh[:, 0:6256], in_=gath_r[:, 0:6256])
                    nc.scalar.dma_start(out=tabh[:, 6256:SH],
                                        in_=gath_r[:, 6256:SH])
                    nc.scalar.activation(table[:, 0:SH, 0], tabh[:, 0:SH],
                                         AF.Copy)

                    # gather + attr-mul + windowed reduce, in big chunks to
                    # amortize the ~13us-per-op gpsimd ISA issue cost
                    for (c0, c1) in chunks:
                        cl = c1 - c0
                        g = hopp.tile([128, GB, 1], dt.float32, tag="g")
                        nc.gpsimd.ap_gather(
                            g[:, 0:cl, :], table[:, 0:SH, :],
                            gidx_sb[:, c0 // 16:c1 // 16],
                            channels=128, num_elems=SH, d=1, num_idxs=cl)
                        nc.vector.tensor_mul(g[:, 0:cl, 0], g[:, 0:cl, 0],
                                             attr_sb[:, c0:c1])
                        for (w, nn, so, lo) in segs:
                            s_end = so + nn * w
                            if s_end <= c0 or so >= c1:
                                continue
                            m0 = (max(c0, so) - so) // w
                            m1 = (min(c1, s_end) - so) // w
                            o = so + m0 * w - c0
                            nc.vector.tensor_reduce(
                                l1out[:, lo + m0:lo + m1, 0],
                                g[:, o:o + (m1 - m0) * w, 0].rearrange(
                                    "p (n w) -> p n w", w=w),
                                axis=mybir.AxisListType.X, op=ALU.add)

                    # realign (rank -> dst order) in big gathers, then
                    # dequant-convert + group-sum matmul + copy out in chunks
                    rc = 0
                    h0 = 0
                    while h0 < SHP:
                        hl = min(GB, SHP - h0)
                        al = hopp.tile([128, GB, 1], dt.float32, tag="g")
                        nc.gpsimd.ap_gather(
                            al[:, 0:hl, :], l1out[:],
                            ridx_sb[:, h0 // 16:(h0 + hl) // 16],
                            channels=128, num_elems=L1N, d=1, num_idxs=hl)
                        for rj in range(hl // RCHUNK):
                            r0 = rc * RCHUNK
                            # dequant-convert on DVE (idle during realign) so
                            # the realign buffer releases without waiting on
                            # the Act/PSUM/staging tail
                            cv = alp.tile([128, RCHUNK], dt.float16, tag="cv")
                            nc.vector.tensor_scalar_mul(
                                cv[:], al[:, rj * RCHUNK:(rj + 1) * RCHUNK, 0],
                                DEQ)
                            ps = hpsum.tile([C, RCHUNK], dt.float32, tag="ps")
                            nc.tensor.matmul(ps[:], sel_sb[:], cv[:],
                                             start=True, stop=True)
                            sg = rc // (STG // RCHUNK)
                            sj = rc % (STG // RCHUNK)
                            if sj == 0:
                                stg = alp.tile([C, STG], dt.float16, tag="stg",
                                               bufs=4)
                            nc.scalar.activation(
                                stg[:, sj * RCHUNK:(sj + 1) * RCHUNK], ps[:],
                                AF.Copy)
                            if sj == STG // RCHUNK - 1:
                                s0 = sg * STG
                                nc.sync.dma_start(
                                    out=pps_d[k][:, s0:s0 + STG], in_=stg[:])
                                if k < K and s0 < SH:
                                    sn = min(STG, SH - s0)
                                    nc.scalar.dma_start(
                                        out=shard_d[:, s0:s0 + sn],
                                        in_=stg[:, 0:sn])
                            rc += 1
                        h0 += hl

        # ---------- combine ----------
        with (
            tc.tile_pool(name="comb", bufs=1) as comb,
            tc.tile_pool(name="combw", bufs=1) as combw,
        ):
            ppsT = comb.tile([128, NSUBP, K + 1, C], dt.float16)
            for k in range(K + 1):
                nc.sync.dma_start_transpose(ppsT[:, :, k, :], pps_d[k][:])
            wk_sb = comb.tile([128, K + 1, C], dt.float32)
            nc.sync.dma_start(out=wk_sb[:], in_=wk_d[:])
            sc_sb = comb.tile([128, K + 1], dt.float32)
            nc.sync.dma_start(out=sc_sb[:], in_=sc_d[:])
            pb_sb = comb.tile([128, 1], dt.float32)
            nc.sync.dma_start(out=pb_sb[:], in_=pb_d[:])

            shape4 = [128, NSUBP, K + 1, C]
            pps32 = comb.tile(shape4, dt.float32)
            nc.vector.tensor_mul(
                pps32[:], ppsT[:],
                sc_sb[:].unsqueeze(1).unsqueeze(3).broadcast_to(shape4))
            prod = combw.tile(shape4, dt.float32, tag="prod")
            nc.vector.tensor_mul(
                prod[:], pps32[:],
                wk_sb[:].unsqueeze(1).broadcast_to(shape4))
            spre = comb.tile([128, NSUBP, K + 1], dt.float32)
            nc.vector.tensor_reduce(spre[:], prod[:], axis=mybir.AxisListType.X,
                                    op=ALU.add)
            score = comb.tile([128, NSUBP, K + 1], dt.float32)
            nc.scalar.activation(score[:], spre[:], AF.Sigmoid, bias=pb_sb[:])
            prod2 = combw.tile(shape4, dt.float32, tag="prod")
            nc.vector.tensor_mul(
                prod2[:], pps32[:],
                score[:].unsqueeze(3).broadcast_to(shape4))
            outsb = comb.tile([128, NSUBP, C], dt.float32)
            nc.vector.tensor_reduce(outsb[:], prod2[:].transpose([0, 1, 3, 2]),
                                    axis=mybir.AxisListType.X, op=ALU.add)
            nc.sync.dma_start(
                out=out_d.rearrange("(a p) c -> p a c", p=128), in_=outsb[:])

    nc.compile()
    return nc


# ----------------------------------------------------------------------------
# Entry point
# ----------------------------------------------------------------------------

def _make_in_maps(pre, x, lin1_w, lin1_b, lin2_w, lin2_b, lin3_w, lin3_b,
                  proj_w, proj_b):
    SH, SHP = _dims(N)
    x = np.asarray(x, np.float32)
    scale16 = (16.0 ** np.arange(K + 1)).astype(np.float32)
    wk = np.broadcast_to(
        np.asarray(proj_w, np.float32)[0][None, None, :], (128, K + 1, C)).copy()
    sc = np.broadcast_to(scale16[None, :], (128, K + 1)).copy()
    pb = np.full((128, 1), np.asarray(proj_b, np.float32)[0], np.float32)
    sel = np.zeros((128, C), f16)
    for p in range(128):
        sel[p, p % C] = 1.0
    common = dict(
        w1t=np.ascontiguousarray(np.asarray(lin1_w, np.float32).T).astype(bf16),
        w2t=np.ascontiguousarray(np.asarray(lin2_w, np.float32).T).astype(bf16),
        w3t=np.ascontiguousarray(np.asarray(lin3_w, np.float32).T).astype(bf16),
        b1=np.asarray(lin1_b, np.float32).reshape(HID, 1),
        b2=np.asarray(lin2_b, np.float32).reshape(HID, 1),
        b3=np.asarray(lin3_b, np.float32).reshape(C, 1),
        ident=np.eye(128, dtype=bf16),
        sel=sel,
        wk=wk, sc=sc, pb=pb,
    )
    in_maps = []
    for c in range(NCORES):
        xp = np.zeros((SHP, F_IN), np.float32)
        xp[:SH] = x[c * SH:(c + 1) * SH]
        in_maps.append(dict(common, xsh=xp,
                            gidx=pre["gidx"][c], ridx=pre["ridx"][c],
                            attr=pre["attr"][c]))
    return in_maps


_CACHE = {}


def _run(trace=False, **inputs):
    from concourse.bass_utils import run_bass_kernel_spmd

    SH, _ = _dims(N)
    pre = _preprocess(inputs["edge_index"], inputs["edge_attr"])
    key = (pre["SLOTS"], pre["L1N"], tuple(pre["segs"]), tuple(pre["chunks"]))
    if key not in _CACHE:
        _CACHE[key] = _build_program(pre["segs"], pre["chunks"], pre["SLOTS"],
                                     pre["L1N"])
    nc = _CACHE[key]

    in_maps = _make_in_maps(
        pre, inputs["x"], inputs["lin1_w"], inputs["lin1_b"],
        inputs["lin2_w"], inputs["lin2_b"], inputs["lin3_w"], inputs["lin3_b"],
        inputs["proj_w"], inputs["proj_b"])
    res = run_bass_kernel_spmd(nc, in_maps, list(range(NCORES)), trace=trace)
    out = np.concatenate([res.results[c]["out"][:SH] for c in range(NCORES)],
                         axis=0)
    return out.astype(np.float32), res


def kernel(x, edge_index, edge_attr, lin1_w, lin1_b, lin2_w, lin2_b,
           lin3_w, lin3_b, proj_w, proj_b):
    out, _ = _run(x=x, edge_index=edge_index, edge_attr=edge_attr,
                  lin1_w=lin1_w, lin1_b=lin1_b, lin2_w=lin2_w, lin2_b=lin2_b,
                  lin3_w=lin3_w, lin3_b=lin3_b, proj_w=proj_w, proj_b=proj_b)
    return out


# revision 23
# speedup vs baseline: 1.0169x; 1.0169x over previous
"""DAGNN (gnn_message_passing) Trainium2 kernel — 8 NeuronCores.

Per core (dst-shard of N/8 nodes):
  * MLP (PE, bf16): x-shard -> h [16, SH] feature-major.
  * K propagation hops:
      - AllGather h shards (f16) -> gather table [128p = 8 src-eighths x 16
        feats, SH] (f32, cast during the table-load DMA)
      - GpSimd ap_gather per slot-segment (edges grouped by src-eighth, laid
        out in degree-sorted windows; width template shared across all
        (core, group) pairs so one tensor_reduce covers all 128 partitions)
      - DVE: multiply by uint8-quantized edge weights (resident in SBUF for
        all hops — no per-hop attr DMA) + windowed tensor_reduce -> L1
        partials (f32)
      - Realign+group-sum pipeline, chunked: GpSimd ap_gather (rank order ->
        dst order), Act scale-convert f32->f16 (dequant 1/(255*16)), PE
        matmul with a 0/1 selection matrix summing the 8 src-eighths,
        Act copy PSUM -> h' [16, SH] f16.
  * Combine: XBAR transpose-load of pps (f16), score = sigmoid(pps @ w + b),
    out = sum_k score_k * pps_k (f32).

h is rescaled by 1/16 per hop (folded into the dequant scale) to keep f16
pps in range; the combine phase multiplies 16^k back in f32.
"""

import math
import os
import sys

import numpy as np

try:
    import concourse  # noqa: F401
except ImportError:
    for _p in ("/opt/trn_rl_repo", "/root/.axon_site/_ro/trn_rl_repo"):
        if os.path.isdir(_p) and _p not in sys.path:
            sys.path.insert(0, _p)

import ml_dtypes

# problem sizes
N = 100000
E = 3200000
F_IN = 256
HID = 128
C = 16
K = 10
NCORES = 8
CHUNK_SLOTS = 3072          # max reduce segment size (slots)
GB = 4928                   # gather chunk scratch size (slots; 11 * RCHUNK)
RCHUNK = 448                # group-sum matmul chunk (<=512 for PSUM bank)
STG = 896                   # staging chunk (2 * RCHUNK)
AQ = 255.0                  # uint8 attr quantization scale

bf16 = ml_dtypes.bfloat16
f16 = np.float16


def _dims(n):
    sh = n // NCORES
    shp = ((sh + 127) // 128) * 128
    return sh, shp


# ----------------------------------------------------------------------------
# Host-side preprocessing
# ----------------------------------------------------------------------------

def _preprocess(edge_index, edge_attr):
    SH, SHP = _dims(N)
    src = np.asarray(edge_index[0], dtype=np.int64)
    dst = np.asarray(edge_index[1], dtype=np.int64)
    attr_q = np.clip(np.rint(np.asarray(edge_attr, np.float64) * AQ), 0,
                     255).astype(np.uint8)

    core_of = dst // SH
    grp_of = src // SH
    dloc = dst - core_of * SH
    sloc = src - grp_of * SH

    cg = (core_of * NCORES + grp_of).astype(np.int64)
    deg = np.zeros((NCORES * NCORES, SH), np.int64)
    np.add.at(deg, (cg, dloc), 1)
    deg = deg.reshape(NCORES, NCORES, SH)

    order = np.argsort(-deg, axis=2, kind="stable")          # [8,8,SH]
    rank = np.empty_like(order)
    ar = np.arange(SH)
    for c in range(NCORES):
        for g in range(NCORES):
            rank[c, g, order[c, g]] = ar
    sdeg = np.take_along_axis(deg, order, axis=2)
    W = np.maximum(sdeg.max(axis=(0, 1)), 1).astype(np.int64)  # non-increasing

    # segments: (w, n_windows, slot_off, l1_off); slot_off%16==0, n*w%16==0
    segs = []
    rank_l1 = np.zeros(SH, np.int64)
    slot_off = 0
    l1_off = 0
    bounds = [0] + list(np.flatnonzero(np.diff(W)) + 1) + [SH]
    for bi in range(len(bounds) - 1):
        i0, i1 = bounds[bi], bounds[bi + 1]
        w = int(W[i0])
        a = 32 // math.gcd(w, 32)
        per = max(a, (CHUNK_SLOTS // (a * w)) * a)
        assert per * w <= max(CHUNK_SLOTS, a * w)
        i = i0
        while i < i1:
            nn = min(per, i1 - i)
            n_pad = ((nn + a - 1) // a) * a
            rank_l1[i:i + nn] = l1_off + np.arange(nn)
            segs.append((w, n_pad, slot_off, l1_off))
            slot_off += n_pad * w
            l1_off += n_pad
            i += nn
    SLOTS = slot_off
    L1N = ((l1_off + 15) // 16) * 16
    assert SLOTS % 32 == 0 and L1N < 32768
    for (_w, _n, _so, _) in segs:
        assert _so % 32 == 0 and (_n * _w) % 32 == 0

    # greedy-pack gather chunks of <= GB slots, cutting inside segments at
    # window boundaries that land on 16-slot alignment
    cuts = [0]
    c0 = 0
    while c0 < SLOTS:
        lim = c0 + GB
        if lim >= SLOTS:
            cuts.append(SLOTS)
            break
        cut = None
        for (w, nn, so, lo) in segs:
            if so >= lim:
                break
            end = so + nn * w
            if end <= lim:
                cut = end
                continue
            # cut inside this segment (32-slot aligned: the gpsimd ucode
            # reads the wrapped idx columns as uint32 pairs)
            a32 = 32 // math.gcd(w, 32)
            m = ((lim - so) // w) // a32 * a32
            if m > 0:
                cut = so + m * w
            break
        assert cut is not None and cut > c0 and cut % 32 == 0
        cuts.append(cut)
        c0 = cut
    chunks = list(zip(cuts[:-1], cuts[1:]))

    seg_w = np.zeros(l1_off, np.int64)
    seg_so = np.zeros(l1_off, np.int64)
    seg_lo = np.zeros(l1_off, np.int64)
    for (w, nn, so, lo) in segs:
        seg_w[lo:lo + nn] = w
        seg_so[lo:lo + nn] = so
        seg_lo[lo:lo + nn] = lo
    slotstart = seg_so[rank_l1] + (rank_l1 - seg_lo[rank_l1]) * seg_w[rank_l1]

    gidx = np.zeros((NCORES, NCORES, SLOTS), np.int16)
    arep = np.zeros((NCORES, NCORES, SLOTS), np.uint8)
    e_sorted = np.argsort(cg, kind="stable")
    bnd = np.searchsorted(cg[e_sorted], np.arange(NCORES * NCORES + 1))
    for c in range(NCORES):
        for g in range(NCORES):
            kk = c * NCORES + g
            idx_e = e_sorted[bnd[kk]:bnd[kk + 1]]
            r = rank[c, g][dloc[idx_e]]
            o = np.argsort(r, kind="stable")
            idx_e, r = idx_e[o], r[o]
            startrun = np.flatnonzero(np.diff(r, prepend=-1))
            runlen = np.diff(np.append(startrun, len(r)))
            within = np.arange(len(r)) - np.repeat(startrun, runlen)
            pos = slotstart[r] + within
            gidx[c, g, pos] = sloc[idx_e].astype(np.int16)
            arep[c, g, pos] = attr_q[idx_e]

    ridx = np.zeros((NCORES, NCORES, SHP), np.int16)
    for c in range(NCORES):
        for g in range(NCORES):
            ridx[c, g, :SH] = rank_l1[rank[c, g]].astype(np.int16)

    def wrap(a):  # [8, L] -> [128, L//16] in ap_gather "(s p)" layout
        L = a.shape[-1]
        return np.ascontiguousarray(
            a.reshape(NCORES, L // 16, 16).transpose(0, 2, 1).reshape(128, L // 16))

    gidx_w = np.stack([wrap(gidx[c]) for c in range(NCORES)])
    ridx_w = np.stack([wrap(ridx[c]) for c in range(NCORES)])
    attr_r = np.repeat(arep[:, :, None, :], C, axis=2).reshape(
        NCORES, 128, SLOTS)

    return dict(segs=segs, chunks=chunks, SLOTS=SLOTS, L1N=L1N,
                gidx=gidx_w, ridx=ridx_w, attr=attr_r)


# ----------------------------------------------------------------------------
# Bass program
# ----------------------------------------------------------------------------

def _build_program(segs, chunks, SLOTS, L1N):
    import concourse.bass as bass
    import concourse.bacc as bacc
    import concourse.tile as tile
    from concourse import mybir

    SH, SHP = _dims(N)
    dt = mybir.dt
    AF = mybir.ActivationFunctionType
    ALU = mybir.AluOpType
    NSUBP = SHP // 128
    NRC = SHP // RCHUNK
    NSTG = SHP // STG
    assert NRC * RCHUNK == SHP and NSTG * STG == SHP
    assert GB % RCHUNK == 0 and SHP % RCHUNK == 0
    DEQ = 1.0 / (AQ * 16.0)

    nc = bacc.Bacc("TRN2", target_bir_lowering=False, debug=False,
                   num_devices=NCORES)

    xsh = nc.declare_dram_parameter("xsh", [SHP, F_IN], dt.float32, isOutput=False)
    w1t = nc.declare_dram_parameter("w1t", [F_IN, HID], dt.bfloat16, isOutput=False)
    w2t = nc.declare_dram_parameter("w2t", [HID, HID], dt.bfloat16, isOutput=False)
    w3t = nc.declare_dram_parameter("w3t", [HID, C], dt.bfloat16, isOutput=False)
    b1 = nc.declare_dram_parameter("b1", [HID, 1], dt.float32, isOutput=False)
    b2 = nc.declare_dram_parameter("b2", [HID, 1], dt.float32, isOutput=False)
    b3 = nc.declare_dram_parameter("b3", [C, 1], dt.float32, isOutput=False)
    ident = nc.declare_dram_parameter("ident", [128, 128], dt.bfloat16, isOutput=False)
    sel_p = nc.declare_dram_parameter("sel", [128, C], dt.float16, isOutput=False)
    gidx_d = nc.declare_dram_parameter("gidx", [128, SLOTS // 16], dt.int16, isOutput=False)
    ridx_d = nc.declare_dram_parameter("ridx", [128, SHP // 16], dt.int16, isOutput=False)
    attr_d = nc.declare_dram_parameter("attr", [128, SLOTS], dt.uint8, isOutput=False)
    wk_d = nc.declare_dram_parameter("wk", [128, K + 1, C], dt.float32, isOutput=False)
    sc_d = nc.declare_dram_parameter("sc", [128, K + 1], dt.float32, isOutput=False)
    pb_d = nc.declare_dram_parameter("pb", [128, 1], dt.float32, isOutput=False)
    out_d = nc.declare_dram_parameter("out", [SHP, C], dt.float32, isOutput=True)

    HLF = 7 * STG               # collective split point (staging-aligned)
    SHB = SH - HLF
    shardA_d = nc.dram_tensor("shardA_hbm", [C, HLF], dt.float16)
    shardB_d = nc.dram_tensor("shardB_hbm", [C, SHB], dt.float16)
    gathA_d = nc.dram_tensor("gathA_hbm", [NCORES, C, HLF], dt.float16,
                             addr_space="Shared")
    gathB_d = nc.dram_tensor("gathB_hbm", [NCORES, C, SHB], dt.float16,
                             addr_space="Shared")
    pps_d = [nc.dram_tensor(f"pps{k}_hbm", [C, SHP], dt.float16)
             for k in range(K + 1)]

    groups = [list(range(NCORES))]

    with tile.TileContext(nc) as tc:
        with tc.tile_pool(name="const", bufs=1) as constp:
            gidx_sb = constp.tile([128, SLOTS // 16], dt.int16)
            nc.sync.dma_start(out=gidx_sb[:], in_=gidx_d[:])
            ridx_sb = constp.tile([128, SHP // 16], dt.int16)
            nc.sync.dma_start(out=ridx_sb[:], in_=ridx_d[:])
            attr_sb = constp.tile([128, SLOTS], dt.uint8)
            nc.scalar.dma_start(out=attr_sb[:], in_=attr_d[:])
            sel_sb = constp.tile([128, C], dt.float16)
            nc.sync.dma_start(out=sel_sb[:], in_=sel_p[:])

            # ---------- MLP ----------
            with (
                tc.tile_pool(name="mlp", bufs=2) as mlpp,
                tc.tile_pool(name="mlpc", bufs=1) as mlpc,
                tc.tile_pool(name="mpsum", bufs=2, space="PSUM") as mpsum,
            ):
                ident_sb = mlpc.tile([128, 128], dt.bfloat16)
                nc.sync.dma_start(out=ident_sb[:], in_=ident[:])
                w1_sb = mlpc.tile([128, 2, HID], dt.bfloat16)
                nc.sync.dma_start(out=w1_sb[:],
                                  in_=w1t.rearrange("(a p) m -> p a m", p=128))
                w2_sb = mlpc.tile([HID, HID], dt.bfloat16)
                nc.sync.dma_start(out=w2_sb[:], in_=w2t[:])
                w3_sb = mlpc.tile([HID, C], dt.bfloat16)
                nc.sync.dma_start(out=w3_sb[:], in_=w3t[:])
                b1_sb = mlpc.tile([HID, 1], dt.float32)
                nc.sync.dma_start(out=b1_sb[:], in_=b1[:])
                b2_sb = mlpc.tile([HID, 1], dt.float32)
                nc.sync.dma_start(out=b2_sb[:], in_=b2[:])
                b3_sb = mlpc.tile([C, 1], dt.float32)
                nc.sync.dma_start(out=b3_sb[:], in_=b3[:])
                hsb = mlpc.tile([C, SHP], dt.float16)

                coff = 0
                ci = 0
                while coff < SHP:
                    cn = min(512, SHP - coff)
                    nsub = cn // 128
                    xin = mlpp.tile([128, nsub, F_IN], dt.float32, tag="xin")
                    eng = nc.sync if ci % 2 == 0 else nc.scalar
                    eng.dma_start(
                        out=xin[:],
                        in_=xsh.rearrange("(a p) f -> p a f", p=128)[
                            :, coff // 128:coff // 128 + nsub, :])
                    xbf = mlpp.tile([128, nsub, F_IN], dt.bfloat16, tag="xbf")
                    nc.vector.tensor_copy(xbf[:], xin[:])
                    xT = mlpp.tile([128, 2, cn], dt.bfloat16, tag="xT")
                    for s in range(nsub):
                        for hlf in range(2):
                            tp = mpsum.tile([128, 128], dt.bfloat16, tag="tp")
                            nc.tensor.transpose(
                                tp[:], xbf[:, s, hlf * 128:(hlf + 1) * 128],
                                ident_sb[:])
                            nc.scalar.activation(
                                xT[:, hlf, s * 128:(s + 1) * 128], tp[:], AF.Copy)
                    h1p = mpsum.tile([HID, cn], dt.float32, tag="h1p")
                    nc.tensor.matmul(h1p[:], w1_sb[:, 0, :], xT[:, 0, :],
                                     start=True, stop=False)
                    nc.tensor.matmul(h1p[:], w1_sb[:, 1, :], xT[:, 1, :],
                                     start=False, stop=True)
                    x2 = mlpp.tile([HID, cn], dt.bfloat16, tag="x2")
                    nc.scalar.activation(x2[:], h1p[:], AF.Relu, bias=b1_sb[:])
                    h2p = mpsum.tile([HID, cn], dt.float32, tag="h1p")
                    nc.tensor.matmul(h2p[:], w2_sb[:], x2[:], start=True, stop=True)
                    x3 = mlpp.tile([HID, cn], dt.bfloat16, tag="x2")
                    nc.scalar.activation(x3[:], h2p[:], AF.Relu, bias=b2_sb[:])
                    h3p = mpsum.tile([C, cn], dt.float32, tag="h3p")
                    nc.tensor.matmul(h3p[:], w3_sb[:], x3[:], start=True, stop=True)
                    nc.scalar.activation(hsb[:, coff:coff + cn], h3p[:], AF.Relu,
                                         bias=b3_sb[:])
                    coff += cn
                    ci += 1
                nc.sync.dma_start(out=pps_d[0][:], in_=hsb[:])
                nc.sync.dma_start(out=shardA_d[:], in_=hsb[:, 0:HLF])
                nc.sync.dma_start(out=shardB_d[:], in_=hsb[:, HLF:SH])

            # ---------- propagation hops ----------
            with (
                tc.tile_pool(name="pers", bufs=1) as pers,
                tc.tile_pool(name="hopp", bufs=2) as hopp,
                tc.tile_pool(name="alp", bufs=2) as alp,
                tc.tile_pool(name="hpsum", bufs=4, space="PSUM") as hpsum,
            ):
                TBP = ((SH + 3) // 4) * 4 + 4
                table = pers.tile([128, TBP, 1], dt.float32)
                l1out = pers.tile([128, L1N, 1], dt.float32)
                # f16 staging aliased onto the upper half of the table buffer;
                # the in-place expanding cast is safe because the f32 write
                # front always trails the f16 read position
                tabh = table[:, TBP // 2:TBP, 0].bitcast(dt.float16)

                gathA_r = gathA_d.rearrange("a b c -> (a b) c")
                gathB_r = gathB_d.rearrange("a b c -> (a b) c")
                for k in range(1, K + 1):
                    # two half-collectives: cc-A only depends on the first 7
                    # staging groups of the previous hop, so it (and its
                    # table half) overlaps the remaining staging and cc-B
                    nc.gpsimd.collective_compute(
                        "AllGather", ALU.bypass, replica_groups=groups,
                        ins=[shardA_d.ap()], outs=[gathA_d.ap()])
                    nc.gpsimd.collective_compute(
                        "AllGather", ALU.bypass, replica_groups=groups,
                        ins=[shardB_d.ap()], outs=[gathB_d.ap()])
                    # raw f16 table load on the HWDGE queues (a casting DMA
                    # would run the f16->f32 convert as a ~130us Q7 software
                    # loop, stalling the gathers); cast on Act instead
                    nc.sync.dma_start(out=tabh[:, 0:HLF], in_=gathA_r[:])
                    nc.scalar.activation(table[:, 0:HLF, 0], tabh[:, 0:HLF],
                                         AF.Copy)
                    nc.scalar.dma_start(out=tabh[:, HLF:SH], in_=gathB_r[:])
                    nc.scalar.activation(table[:, HLF:SH, 0], tabh[:, HLF:SH],
                                         AF.Copy)

                    # gather + attr-mul + windowed reduce, in big chunks to
                    # amortize the ~13us-per-op gpsimd ISA issue cost
                    for (c0, c1) in chunks:
                        cl = c1 - c0
                        g = hopp.tile([128, GB, 1], dt.float32, tag="g")
                        nc.gpsimd.ap_gather(
                            g[:, 0:cl, :], table[:, 0:SH, :],
                            gidx_sb[:, c0 // 16:c1 // 16],
                            channels=128, num_elems=SH, d=1, num_idxs=cl)
                        nc.vector.tensor_mul(g[:, 0:cl, 0], g[:, 0:cl, 0],
                                             attr_sb[:, c0:c1])
                        for (w, nn, so, lo) in segs:
                            s_end = so + nn * w
                            if s_end <= c0 or so >= c1:
                                continue
                            m0 = (max(c0, so) - so) // w
                            m1 = (min(c1, s_end) - so) // w
                            o = so + m0 * w - c0
                            nc.vector.tensor_reduce(
                                l1out[:, lo + m0:lo + m1, 0],
                                g[:, o:o + (m1 - m0) * w, 0].rearrange(
                                    "p (n w) -> p n w", w=w),
                                axis=mybir.AxisListType.X, op=ALU.add)

                    # realign (rank -> dst order) in big gathers, then
                    # dequant-convert + group-sum matmul + copy out in chunks
                    rc = 0
                    h0 = 0
                    while h0 < SHP:
                        hl = min(GB, SHP - h0)
                        al = hopp.tile([128, GB, 1], dt.float32, tag="g")
                        nc.gpsimd.ap_gather(
                            al[:, 0:hl, :], l1out[:],
                            ridx_sb[:, h0 // 16:(h0 + hl) // 16],
                            channels=128, num_elems=L1N, d=1, num_idxs=hl)
                        for rj in range(hl // RCHUNK):
                            r0 = rc * RCHUNK
                            # dequant-convert on DVE (idle during realign) so
                            # the realign buffer releases without waiting on
                            # the Act/PSUM/staging tail
                            cv = alp.tile([128, RCHUNK], dt.float16, tag="cv")
                            nc.vector.tensor_scalar_mul(
                                cv[:], al[:, rj * RCHUNK:(rj + 1) * RCHUNK, 0],
                                DEQ)
                            ps = hpsum.tile([C, RCHUNK], dt.float32, tag="ps")
                            nc.tensor.matmul(ps[:], sel_sb[:], cv[:],
                                             start=True, stop=True)
                            sg = rc // (STG // RCHUNK)
                            sj = rc % (STG // RCHUNK)
                            if sj == 0:
                                stg = alp.tile([C, STG], dt.float16, tag="stg",
                                               bufs=4)
                            nc.scalar.activation(
                                stg[:, sj * RCHUNK:(sj + 1) * RCHUNK], ps[:],
                                AF.Copy)
                            if sj == STG // RCHUNK - 1:
                                s0 = sg * STG
                                nc.sync.dma_start(
                                    out=pps_d[k][:, s0:s0 + STG], in_=stg[:])
                                if k < K and s0 < SH:
                                    sn = min(STG, SH - s0)
                                    if s0 < HLF:
                                        nc.scalar.dma_start(
                                            out=shardA_d[:, s0:s0 + sn],
                                            in_=stg[:, 0:sn])
                                    else:
                                        nc.scalar.dma_start(
                                            out=shardB_d[:, s0 - HLF:
                                                          s0 - HLF + sn],
                                            in_=stg[:, 0:sn])
                            rc += 1
                        h0 += hl

        # ---------- combine ----------
        with (
            tc.tile_pool(name="comb", bufs=1) as comb,
            tc.tile_pool(name="combw", bufs=1) as combw,
        ):
            ppsT = comb.tile([128, NSUBP, K + 1, C], dt.float16)
            for k in range(K + 1):
                nc.sync.dma_start_transpose(ppsT[:, :, k, :], pps_d[k][:])
            wk_sb = comb.tile([128, K + 1, C], dt.float32)
            nc.sync.dma_start(out=wk_sb[:], in_=wk_d[:])
            sc_sb = comb.tile([128, K + 1], dt.float32)
            nc.sync.dma_start(out=sc_sb[:], in_=sc_d[:])
            pb_sb = comb.tile([128, 1], dt.float32)
            nc.sync.dma_start(out=pb_sb[:], in_=pb_d[:])

            shape4 = [128, NSUBP, K + 1, C]
            pps32 = comb.tile(shape4, dt.float32)
            nc.vector.tensor_mul(
                pps32[:], ppsT[:],
                sc_sb[:].unsqueeze(1).unsqueeze(3).broadcast_to(shape4))
            prod = combw.tile(shape4, dt.float32, tag="prod")
            nc.vector.tensor_mul(
                prod[:], pps32[:],
                wk_sb[:].unsqueeze(1).broadcast_to(shape4))
            spre = comb.tile([128, NSUBP, K + 1], dt.float32)
            nc.vector.tensor_reduce(spre[:], prod[:], axis=mybir.AxisListType.X,
                                    op=ALU.add)
            score = comb.tile([128, NSUBP, K + 1], dt.float32)
            nc.scalar.activation(score[:], spre[:], AF.Sigmoid, bias=pb_sb[:])
            prod2 = combw.tile(shape4, dt.float32, tag="prod")
            nc.vector.tensor_mul(
                prod2[:], pps32[:],
                score[:].unsqueeze(3).broadcast_to(shape4))
            outsb = comb.tile([128, NSUBP, C], dt.float32)
            nc.vector.tensor_reduce(outsb[:], prod2[:].transpose([0, 1, 3, 2]),
                                    axis=mybir.AxisListType.X, op=ALU.add)
            nc.sync.dma_start(
                out=out_d.rearrange("(a p) c -> p a c", p=128), in_=outsb[:])

    nc.compile()
    return nc


# ----------------------------------------------------------------------------
# Entry point
# ----------------------------------------------------------------------------

def _make_in_maps(pre, x, lin1_w, lin1_b, lin2_w, lin2_b, lin3_w, lin3_b,
                  proj_w, proj_b):
    SH, SHP = _dims(N)
    x = np.asarray(x, np.float32)
    scale16 = (16.0 ** np.arange(K + 1)).astype(np.float32)
    wk = np.broadcast_to(
        np.asarray(proj_w, np.float32)[0][None, None, :], (128, K + 1, C)).copy()
    sc = np.broadcast_to(scale16[None, :], (128, K + 1)).copy()
    pb = np.full((128, 1), np.asarray(proj_b, np.float32)[0], np.float32)
    sel = np.zeros((128, C), f16)
    for p in range(128):
        sel[p, p % C] = 1.0
    common = dict(
        w1t=np.ascontiguousarray(np.asarray(lin1_w, np.float32).T).astype(bf16),
        w2t=np.ascontiguousarray(np.asarray(lin2_w, np.float32).T).astype(bf16),
        w3t=np.ascontiguousarray(np.asarray(lin3_w, np.float32).T).astype(bf16),
        b1=np.asarray(lin1_b, np.float32).reshape(HID, 1),
        b2=np.asarray(lin2_b, np.float32).reshape(HID, 1),
        b3=np.asarray(lin3_b, np.float32).reshape(C, 1),
        ident=np.eye(128, dtype=bf16),
        sel=sel,
        wk=wk, sc=sc, pb=pb,
    )
    in_maps = []
    for c in range(NCORES):
        xp = np.zeros((SHP, F_IN), np.float32)
        xp[:SH] = x[c * SH:(c + 1) * SH]
        in_maps.append(dict(common, xsh=xp,
                            gidx=pre["gidx"][c], ridx=pre["ridx"][c],
                            attr=pre["attr"][c]))
    return in_maps


_CACHE = {}


def _run(trace=False, **inputs):
    from concourse.bass_utils import run_bass_kernel_spmd

    SH, _ = _dims(N)
    pre = _preprocess(inputs["edge_index"], inputs["edge_attr"])
    key = (pre["SLOTS"], pre["L1N"], tuple(pre["segs"]), tuple(pre["chunks"]))
    if key not in _CACHE:
        _CACHE[key] = _build_program(pre["segs"], pre["chunks"], pre["SLOTS"],
                                     pre["L1N"])
    nc = _CACHE[key]

    in_maps = _make_in_maps(
        pre, inputs["x"], inputs["lin1_w"], inputs["lin1_b"],
        inputs["lin2_w"], inputs["lin2_b"], inputs["lin3_w"], inputs["lin3_b"],
        inputs["proj_w"], inputs["proj_b"])
    res = run_bass_kernel_spmd(nc, in_maps, list(range(NCORES)), trace=trace)
    out = np.concatenate([res.results[c]["out"][:SH] for c in range(NCORES)],
                         axis=0)
    return out.astype(np.float32), res


def kernel(x, edge_index, edge_attr, lin1_w, lin1_b, lin2_w, lin2_b,
           lin3_w, lin3_b, proj_w, proj_b):
    out, _ = _run(x=x, edge_index=edge_index, edge_attr=edge_attr,
                  lin1_w=lin1_w, lin1_b=lin1_b, lin2_w=lin2_w, lin2_b=lin2_b,
                  lin3_w=lin3_w, lin3_b=lin3_b, proj_w=proj_w, proj_b=proj_b)
    return out
